# revision 26
# baseline (speedup 1.0000x reference)
"""Trainium2 Bass kernel for equivariant multihead attention (v3).

Math (per batch b, query point i, coset s1, channel c):
    logit[j,s2] = sum_g pairwise_g[b,i,j,s1,s2,g]*w_g[c,g]
                  + w_y[c,0]*y[b,j,s2,c] + w_y[c,1]*y[b,i,s1,c] + b_g[c] + b_y[c]
    att = exp(logit)*mask[b,j,s2];  att /= sum_{j,s2} att
    out = (y[b,i,s1,c] + sum_{j,s2} att*y[b,j,s2,c]) * mask[b,i,s1]  @ w_lin.T

Query-side terms and biases are constant over the key dims (j,s2) and cancel
in the normalization -> dropped.  The key-side factor is folded into the
exponent: lkd[c,key] = w_y[c,0]*y[b,j,s2,c]  (masked keys are dropped
entirely on host -- key compaction -- so no log-mask term is needed; padding
keys up to a multiple of 32 carry lkd=-80 -> exp ~ 0).

Layout: one PE matmul pair per PAIR of query blocks ("unit") computes the
complete biased logits.  PSUM rows m = (blk2, s1, c) = 128, free cols =
compacted keys (K_b of 1024, per batch; two bank-halves).  Contraction rows
k = (blk2, s1, g) = 112 rows of pairwise data plus 8 indicator rows that add
lkd[c] -> K = 120.  Then per unit
    E = exp(PSUM) in place     (ACT, one instr; accum_out gives den for free)
    num = sum_keys E * y_tab   (DVE, one scalar_tensor_tensor with accum)
and den/num land in two columns of a [128, 64] accumulator that is DMA'd out
(pipelined).  Host finishes with residual add, query mask and the c_in->c_out
linear.  Steady state is ACT-bound (~36us busy) with DVE ~35us right behind;
den-on-ACT-accum (187ns aux) vs den-on-DVE (a ~1.1us tensor_reduce) and
STT's fused multiply+reduce make this the balanced optimum -- DVE reduce ops
have no 16-bit fast modes, and Pool/gpsimd cannot run STT at all.

All matmul operands are bf16 (tolerance is 2e-2; bf16 keeps PE at 1 cyc/row
and halves HBM traffic); exp/accumulations are f32.

Sharding: query dim i is split 8 ways (16 i x 4 b = 64 blocks = 32 units/core).
"""

import numpy as np
import ml_dtypes

import concourse.bacc as bacc
import concourse.tile as tile
from concourse import mybir
from concourse.bass_utils import run_bass_kernel_spmd

B, N, S, CIN, COUT, GDIM = 4, 128, 8, 8, 8, 7
NCORES = 8
ISHARD = N // NCORES          # 16 query points per core
NBLK = B * ISHARD             # 64 (b,i) blocks per core
NUNIT = NBLK // 2             # 32 units of 2 blocks
UPB = NUNIT // B              # 8 units per batch
KEYW = S * N                  # 1024 key columns before compaction
KROWS = 2 * S * GDIM + CIN    # 120 contraction rows (112 pairwise + 8 lkd)

# DMA schedule after the first (W2 + unit 0) DMA: ("u", a, b) loads unit
# tiles [a, b), ("y", a, b) loads y_tab batches [a, b).  Sized so each
# transfer's completion lands just before compute needs it (compute drains
# ~1.2us/unit; DMA delivers ~0.8us/unit serially).
DMA_ITEMS = (("u", 1, 2), ("y", 0, 1), ("u", 2, 3), ("u", 3, 5),
             ("u", 5, 7), ("y", 1, 2), ("u", 7, 11), ("u", 11, 16),
             ("y", 2, 3), ("u", 16, 22), ("u", 22, 28), ("y", 3, 4),
             ("u", 28, 32))

F32 = mybir.dt.float32
BF16 = mybir.dt.bfloat16
NPBF16 = ml_dtypes.bfloat16

_PROGRAM_CACHE = {}
_LAST_WIDTHS = (KEYW,) * B    # per-batch padded key counts from _host_prep
INPLACE_EXP = True            # exp PSUM->PSUM in place vs via SBUF bf16 tile


def _layout(widths):
    """Blob column layout for per-batch key widths: y_tab | W2 | unit tiles."""
    yoff = [0]
    for w in widths:
        yoff.append(yoff[-1] + w)
    w2c = yoff[-1]
    u0 = w2c + 128
    ucol = [u0]
    for u in range(NUNIT):
        ucol.append(ucol[-1] + widths[u // UPB])
    return yoff, w2c, ucol


def _build_program(nblk=NBLK, loop_reps=1, widths=None):
    """loop_reps>1 wraps the main loop in a hardware For_i that re-runs the
    full pass (including the input DMAs) on the same data -- used only for
    timing: wall(loop_reps=R) - wall(loop_reps=1) isolates device time from
    the ~100ms axon dispatch/transfer overhead."""
    if widths is None:
        widths = (KEYW,) * B
    nunit = nblk // 2
    yoff, w2c, ucol = _layout(widths)
    totw = ucol[nunit]

    nc = bacc.Bacc("TRN2", target_bir_lowering=False, debug=False,
                   num_devices=NCORES)
    blob_d = nc.dram_tensor("blob16", (128, totw), BF16,
                            kind="ExternalInput").ap()
    out_s = nc.dram_tensor("out_s", (128, 2 * NUNIT), F32,
                           kind="ExternalOutput").ap()

    items = [it for it in DMA_ITEMS
             if it[0] == "y" or it[1] < nunit]
    items = [(t, a, min(b_, nunit)) if t == "u" else (t, a, b_)
             for (t, a, b_) in items]

    inplace = INPLACE_EXP
    with tile.TileContext(nc) as tc:
        with (
            tc.tile_pool(name="consts", bufs=1) as consts,
            tc.tile_pool(name="epool", bufs=4) as epool,
            tc.tile_pool(name="psum", bufs=3, space="PSUM") as psum,
        ):
            g16 = consts.tile([128, totw], BF16)
            w2 = g16[0:KROWS, w2c:w2c + 128]
            acc = consts.tile([128, 2 * NUNIT], F32)

            NDUM = 8
            dummies = [consts.tile([128, 1], BF16, name=f"dum{i}")
                       for i in range(NDUM)]

            def main_pass():
                # W2 + first half of unit 0, then the rest of unit 0, so the
                # first matmul can start as early as possible
                mid0 = ucol[0] + widths[0] // 2
                nc.sync.dma_start(g16[0:KROWS, w2c:mid0],
                                  blob_d[0:KROWS, w2c:mid0])
                nc.sync.dma_start(g16[0:KROWS, mid0:ucol[1]],
                                  blob_d[0:KROWS, mid0:ucol[1]])
                for (t, a, b_) in items:
                    if t == "u":
                        c0, c1 = ucol[a], ucol[b_]
                        nc.sync.dma_start(g16[0:KROWS, c0:c1],
                                          blob_d[0:KROWS, c0:c1])
                    else:
                        c0, c1 = yoff[a], yoff[b_]
                        nc.sync.dma_start(g16[:, c0:c1], blob_d[:, c0:c1])
                for u in range(nunit):
                    b = u // UPB
                    kw = widths[b]
                    kh = kw // 2
                    pt = psum.tile([128, 2, 512], F32, tag="l")
                    for h in range(2):
                        nc.tensor.matmul(
                            pt[:, h, 0:kh],
                            lhsT=w2,
                            rhs=g16[0:KROWS, ucol[u] + h * kh:
                                    ucol[u] + (h + 1) * kh],
                            start=True, stop=True)
                    ytab = g16[:, yoff[b]:yoff[b] + kw].rearrange(
                        "p (h k) -> p h k", h=2)
                    dum = dummies[u % NDUM]
                    if inplace:
                        e_ap = pt[:, :, 0:kh]
                    else:
                        e_t = epool.tile([128, KEYW], BF16, tag="e")
                        e_ap = e_t[:, 0:kw].rearrange("p (h k) -> p h k", h=2)
                    nc.scalar.activation(
                        e_ap, pt[:, :, 0:kh],
                        mybir.ActivationFunctionType.Exp,
                        accum_out=acc[:, u:u + 1])
                    nc.vector.scalar_tensor_tensor(
                        dum.broadcast_to((128, 2, kh)), e_ap, 0.0, ytab,
                        op0=mybir.AluOpType.bypass, op1=mybir.AluOpType.mult,
                        accum_out=acc[:, NUNIT + u:NUNIT + u + 1])

            if loop_reps > 1:
                with tc.For_i(0, loop_reps, 1,
                              hint_engines=(mybir.EngineType.PE,
                                            mybir.EngineType.Activation,
                                            mybir.EngineType.DVE,
                                            mybir.EngineType.SP)):
                    main_pass()
            else:
                main_pass()

            # pipeline the result out: everything but the last unit's two
            # columns ships while unit 31 still computes; the final DMA
            # moves just those two (strided) columns
            last = nunit - 1
            nc.sync.dma_start(out_s[:, 0:last], acc[:, 0:last])
            nc.sync.dma_start(out_s[:, nunit:nunit + last],
                              acc[:, nunit:nunit + last])
            nc.sync.dma_start(out_s[:, last:2 * nunit:nunit],
                              acc[:, last:2 * nunit:nunit])

    nc.compile()   # bacc: register alloc + split_sync_waits (1-wait limit)
    return nc


def _get_program(nblk=NBLK, loop_reps=1, widths=None):
    if widths is None:
        widths = _LAST_WIDTHS
    key = ("nc", nblk, loop_reps, widths, INPLACE_EXP)
    if key not in _PROGRAM_CACHE:
        _PROGRAM_CACHE[key] = _build_program(nblk, loop_reps, widths)
    return _PROGRAM_CACHE[key]


def _host_prep(pairwise_g, coset_functions, mask, w_y, w_g):
    """Build the per-core bf16 input blobs (keys compacted per batch)."""
    global _LAST_WIDTHS
    y = coset_functions.astype(np.float32)          # (B, N, S, C) keys
    maskb = np.asarray(mask, bool)

    # compacted key lists per batch over (s2, j)
    mkey = maskb.transpose(0, 2, 1).reshape(B, KEYW)      # [b, (s2,j)]
    idxs, widths = [], []
    for b in range(B):
        idx = np.flatnonzero(mkey[b])
        idxs.append(idx)
        widths.append(max(2, (len(idx) + 1) // 2 * 2))
    widths = tuple(widths)
    _LAST_WIDTHS = widths
    yoff, w2c, ucol = _layout(widths)
    totw = ucol[NUNIT]

    # lkd[b, c, (s2, j)] = w_y[c,0]*y[b,j,s2,c]
    yT = y.transpose(0, 3, 2, 1)                    # (B, C, S, N) = [b,c,s2,j]
    lkd = (w_y[:, 0][None, :, None, None] * yT).reshape(B, CIN, KEYW)
    ytabf = np.tile(yT.reshape(B, CIN, KEYW), (1, 128 // CIN, 1))  # [b,128,K]

    # W2 [KROWS, 128]: col m = (blk2, s1, c)
    w2 = np.zeros((KROWS, 128), np.float32)
    for blk2 in range(2):
        for s1 in range(S):
            for g in range(GDIM):
                for c in range(CIN):
                    w2[blk2 * 56 + s1 * GDIM + g,
                       blk2 * 64 + s1 * CIN + c] = w_g[c, g]
    for blk2 in range(2):
        for s1 in range(S):
            for c in range(CIN):
                w2[2 * S * GDIM + c, blk2 * 64 + s1 * CIN + c] = 1.0

    in_maps = []
    for k in range(NCORES):
        sl = slice(ISHARD * k, ISHARD * (k + 1))
        pg = pairwise_g[:, sl]                      # (B, 16, N, S, S, G)
        pgr = pg.reshape(B, UPB, 2, N, S, S, GDIM)  # [b,iu,blk2,j,s1,s2,g]
        pgr = pgr.transpose(0, 1, 2, 4, 6, 5, 3)    # [b,iu,blk2,s1,g,s2,j]
        pgr = pgr.reshape(B, UPB, 112, KEYW)

        blob = np.zeros((128, totw), NPBF16)
        blob[0:KROWS, w2c:w2c + 128] = w2
        for b in range(B):
            idx, kw = idxs[b], widths[b]
            nk = len(idx)
            blob[:, yoff[b]:yoff[b] + nk] = ytabf[b][:, idx]
            for iu in range(UPB):
                u = b * UPB + iu
                c0 = ucol[u]
                blob[0:112, c0:c0 + nk] = pgr[b, iu][:, idx]
                blob[112:KROWS, c0:c0 + nk] = lkd[b][:, idx]
                if nk < kw:
                    blob[112:KROWS, c0 + nk:c0 + kw] = -80.0
        in_maps.append({"blob16": blob})
    return in_maps


def _host_finish(s_list, coset_functions, mask, w_lin):
    """Decode per-core (128, 64) den|num columns into the full result."""
    y = np.asarray(coset_functions, dtype=np.float32)
    maskf = np.asarray(mask).astype(np.float32)
    out = np.empty((B, N, S, COUT), np.float32)
    for k in range(NCORES):
        s = np.asarray(s_list[k], np.float32)
        den = s[:, :NUNIT].reshape(2, S, CIN, B, UPB)  # [blk2, s1, c, b, iu]
        num = s[:, NUNIT:].reshape(2, S, CIN, B, UPB)
        den = den.transpose(3, 4, 0, 1, 2).reshape(B, ISHARD, S, CIN)
        num = num.transpose(3, 4, 0, 1, 2).reshape(B, ISHARD, S, CIN)
        sl = slice(ISHARD * k, ISHARD * (k + 1))
        y_q = y[:, sl]
        m_q = maskf[:, sl]
        res = (y_q + num / den) * m_q[..., None]
        out[:, sl] = res @ w_lin.T
    return out


def kernel(pairwise_g, coset_functions, mask, w_y, b_y, w_g, b_g, w_lin):
    pairwise_g = np.asarray(pairwise_g, dtype=np.float32)
    coset_functions = np.asarray(coset_functions, dtype=np.float32)
    mask = np.asarray(mask)
    w_y = np.asarray(w_y, dtype=np.float32)
    w_g = np.asarray(w_g, dtype=np.float32)
    w_lin = np.asarray(w_lin, dtype=np.float32)

    in_maps = _host_prep(pairwise_g, coset_functions, mask, w_y, w_g)
    nc = _get_program()
    res = run_bass_kernel_spmd(nc, in_maps, core_ids=list(range(NCORES)))
    s_list = [r["out_s"] for r in res.results]
    return _host_finish(s_list, coset_functions, mask, w_lin)


# revision 27
# speedup vs baseline: 1.2170x; 1.2170x over previous
"""Trainium2 Bass kernel for equivariant multihead attention (v3).

Math (per batch b, query point i, coset s1, channel c):
    logit[j,s2] = sum_g pairwise_g[b,i,j,s1,s2,g]*w_g[c,g]
                  + w_y[c,0]*y[b,j,s2,c] + w_y[c,1]*y[b,i,s1,c] + b_g[c] + b_y[c]
    att = exp(logit)*mask[b,j,s2];  att /= sum_{j,s2} att
    out = (y[b,i,s1,c] + sum_{j,s2} att*y[b,j,s2,c]) * mask[b,i,s1]  @ w_lin.T

Query-side terms and biases are constant over the key dims (j,s2) and cancel
in the normalization -> dropped.  The key-side factor is folded into the
exponent: lkd[c,key] = w_y[c,0]*y[b,j,s2,c]  (masked keys are dropped
entirely on host -- key compaction -- so no log-mask term is needed; padding
keys up to an even count carry lkd=-80 -> exp ~ 0).

Layout: one PE matmul pair per PAIR of query blocks ("unit") computes the
complete biased logits.  PSUM rows m = (blk2, s1, c) = 128, free cols =
compacted keys (K_b of 1024, per batch; two bank-halves).  Contraction rows
k = (blk2, s1, g) = 112 rows of pairwise data plus 8 indicator rows that add
lkd[c] -> K = 120.  Then per unit
    E = exp(PSUM) in place     (ACT, one instr; accum_out gives den for free)
    num = sum_keys E * y_tab   (DVE, one scalar_tensor_tensor with accum)
and den/num land in two columns of a [128, 64] accumulator that is DMA'd out
(pipelined).  Host finishes with residual add, query mask and the c_in->c_out
linear.  Steady state is ACT-bound (~36us busy) with DVE ~35us right behind;
den-on-ACT-accum (187ns aux) vs den-on-DVE (a ~1.1us tensor_reduce) and
STT's fused multiply+reduce make this the balanced optimum -- DVE reduce ops
have no 16-bit fast modes, and Pool/gpsimd cannot run STT at all.

All matmul operands are bf16 (tolerance is 2e-2; bf16 keeps PE at 1 cyc/row
and halves HBM traffic); exp/accumulations are f32.

Sharding: query dim i is split 8 ways (16 i x 4 b = 64 blocks = 32 units/core).
"""

import numpy as np
import ml_dtypes

import concourse.bacc as bacc
import concourse.tile as tile
from concourse import mybir
from concourse.bass_utils import run_bass_kernel_spmd

B, N, S, CIN, COUT, GDIM = 4, 128, 8, 8, 8, 7
NCORES = 8
ISHARD = N // NCORES          # 16 query points per core
NBLK = B * ISHARD             # 64 (b,i) blocks per core
NUNIT = NBLK // 2             # 32 units of 2 blocks
UPB = NUNIT // B              # 8 units per batch
KEYW = S * N                  # 1024 key columns before compaction
KROWS = 2 * S * GDIM + CIN    # 120 contraction rows (112 pairwise + 8 lkd)

# DMA schedule after the first (W2 + unit 0) DMA: ("u", a, b) loads unit
# tiles [a, b), ("y", a, b) loads y_tab batches [a, b).  Sized so each
# transfer's completion lands just before compute needs it (compute drains
# ~1.2us/unit; DMA delivers ~0.8us/unit serially).
DMA_ITEMS = (("u", 1, 2), ("y", 0, 1), ("u", 2, 3), ("u", 3, 5),
             ("u", 5, 7), ("y", 1, 2), ("u", 7, 11), ("u", 11, 16),
             ("y", 2, 3), ("u", 16, 22), ("u", 22, 28), ("y", 3, 4),
             ("u", 28, 32))

F32 = mybir.dt.float32
BF16 = mybir.dt.bfloat16
NPBF16 = ml_dtypes.bfloat16

_PROGRAM_CACHE = {}
_LAST_WIDTHS = (KEYW,) * B    # per-batch padded key counts from _host_prep
INPLACE_EXP = True            # exp PSUM->PSUM in place vs via SBUF bf16 tile


def _layout(widths):
    """Blob column layout for per-batch key widths: y_tab | W2 | unit tiles."""
    yoff = [0]
    for w in widths:
        yoff.append(yoff[-1] + w)
    w2c = yoff[-1]
    u0 = w2c + 128
    ucol = [u0]
    for u in range(NUNIT):
        ucol.append(ucol[-1] + widths[u // UPB])
    return yoff, w2c, ucol


def _build_program(nblk=NBLK, loop_reps=1, widths=None):
    """loop_reps>1 wraps the main loop in a hardware For_i that re-runs the
    full pass (including the input DMAs) on the same data -- used only for
    timing: wall(loop_reps=R) - wall(loop_reps=1) isolates device time from
    the ~100ms axon dispatch/transfer overhead."""
    if widths is None:
        widths = (KEYW,) * B
    nunit = nblk // 2
    yoff, w2c, ucol = _layout(widths)
    totw = ucol[nunit]

    nc = bacc.Bacc("TRN2", target_bir_lowering=False, debug=False,
                   num_devices=NCORES)
    blob_d = nc.dram_tensor("blob16", (128, totw), BF16,
                            kind="ExternalInput").ap()
    out_s = nc.dram_tensor("out_s", (128, 2 * NUNIT), F32,
                           kind="ExternalOutput").ap()

    items = [it for it in DMA_ITEMS
             if it[0] == "y" or it[1] < nunit]
    items = [(t, a, min(b_, nunit)) if t == "u" else (t, a, b_)
             for (t, a, b_) in items]

    inplace = INPLACE_EXP
    with tile.TileContext(nc) as tc:
        with (
            tc.tile_pool(name="consts", bufs=1) as consts,
            tc.tile_pool(name="epool", bufs=4) as epool,
            tc.tile_pool(name="psum", bufs=3, space="PSUM") as psum,
        ):
            g16 = consts.tile([128, totw], BF16)
            w2 = g16[0:KROWS, w2c:w2c + 128]
            acc = consts.tile([128, 2 * NUNIT], F32)

            NDUM = 8
            dummies = [consts.tile([128, 1], BF16, name=f"dum{i}")
                       for i in range(NDUM)]

            def main_pass():
                # W2 + first half of unit 0, then the rest of unit 0, so the
                # first matmul can start as early as possible
                mid0 = ucol[0] + widths[0] // 2
                nc.sync.dma_start(g16[0:KROWS, w2c:mid0],
                                  blob_d[0:KROWS, w2c:mid0])
                nc.sync.dma_start(g16[0:KROWS, mid0:ucol[1]],
                                  blob_d[0:KROWS, mid0:ucol[1]])
                for (t, a, b_) in items:
                    if t == "u":
                        c0, c1 = ucol[a], ucol[b_]
                        nc.sync.dma_start(g16[0:KROWS, c0:c1],
                                          blob_d[0:KROWS, c0:c1])
                    else:
                        c0, c1 = yoff[a], yoff[b_]
                        nc.sync.dma_start(g16[:, c0:c1], blob_d[:, c0:c1])
                for u in range(nunit):
                    b = u // UPB
                    kw = widths[b]
                    kh = kw // 2
                    pt = psum.tile([128, 2, 512], F32, tag="l")
                    for h in range(2):
                        nc.tensor.matmul(
                            pt[:, h, 0:kh],
                            lhsT=w2,
                            rhs=g16[0:KROWS, ucol[u] + h * kh:
                                    ucol[u] + (h + 1) * kh],
                            start=True, stop=True)
                    ytab = g16[:, yoff[b]:yoff[b] + kw].rearrange(
                        "p (h k) -> p h k", h=2)
                    dum = dummies[u % NDUM]
                    if inplace:
                        e_ap = pt[:, :, 0:kh]
                    else:
                        e_t = epool.tile([128, KEYW], BF16, tag="e")
                        e_ap = e_t[:, 0:kw].rearrange("p (h k) -> p h k", h=2)
                    nc.scalar.activation(
                        e_ap, pt[:, :, 0:kh],
                        mybir.ActivationFunctionType.Exp,
                        accum_out=acc[:, u:u + 1])
                    nc.vector.scalar_tensor_tensor(
                        dum.broadcast_to((128, 2, kh)), e_ap, 0.0, ytab,
                        op0=mybir.AluOpType.bypass, op1=mybir.AluOpType.mult,
                        accum_out=acc[:, NUNIT + u:NUNIT + u + 1])

            if loop_reps > 1:
                with tc.For_i(0, loop_reps, 1,
                              hint_engines=(mybir.EngineType.PE,
                                            mybir.EngineType.Activation,
                                            mybir.EngineType.DVE,
                                            mybir.EngineType.SP)):
                    main_pass()
            else:
                main_pass()

            # pipeline the result out: everything but the last unit's two
            # columns ships while unit 31 still computes; the final DMA
            # moves just those two (strided) columns
            last = nunit - 1
            nc.sync.dma_start(out_s[:, 0:last], acc[:, 0:last])
            nc.sync.dma_start(out_s[:, nunit:nunit + last],
                              acc[:, nunit:nunit + last])
            nc.sync.dma_start(out_s[:, last:2 * nunit:nunit],
                              acc[:, last:2 * nunit:nunit])

    nc.compile()   # bacc: register alloc + split_sync_waits (1-wait limit)
    return nc


def _get_program(nblk=NBLK, loop_reps=1, widths=None):
    if widths is None:
        widths = _LAST_WIDTHS
    key = ("nc", nblk, loop_reps, widths, INPLACE_EXP)
    if key not in _PROGRAM_CACHE:
        _PROGRAM_CACHE[key] = _build_program(nblk, loop_reps, widths)
    return _PROGRAM_CACHE[key]


def _host_prep(pairwise_g, coset_functions, mask, w_y, w_g):
    """Build the per-core bf16 input blobs (keys compacted per batch)."""
    global _LAST_WIDTHS
    y = coset_functions.astype(np.float32)          # (B, N, S, C) keys
    maskb = np.asarray(mask, bool)

    # compacted key lists per batch over (s2, j)
    mkey = maskb.transpose(0, 2, 1).reshape(B, KEYW)      # [b, (s2,j)]
    idxs, widths = [], []
    for b in range(B):
        idx = np.flatnonzero(mkey[b])
        idxs.append(idx)
        widths.append(max(2, (len(idx) + 1) // 2 * 2))
    widths = tuple(widths)
    _LAST_WIDTHS = widths
    yoff, w2c, ucol = _layout(widths)
    totw = ucol[NUNIT]

    # lkd[b, c, (s2, j)] = w_y[c,0]*y[b,j,s2,c]
    yT = y.transpose(0, 3, 2, 1)                    # (B, C, S, N) = [b,c,s2,j]
    lkd = (w_y[:, 0][None, :, None, None] * yT).reshape(B, CIN, KEYW)
    ytabf = np.tile(yT.reshape(B, CIN, KEYW), (1, 128 // CIN, 1))  # [b,128,K]

    # W2 [KROWS, 128]: col m = (blk2, s1, c)
    w2 = np.zeros((KROWS, 128), np.float32)
    for blk2 in range(2):
        for s1 in range(S):
            for g in range(GDIM):
                for c in range(CIN):
                    w2[blk2 * 56 + s1 * GDIM + g,
                       blk2 * 64 + s1 * CIN + c] = w_g[c, g]
    for blk2 in range(2):
        for s1 in range(S):
            for c in range(CIN):
                w2[2 * S * GDIM + c, blk2 * 64 + s1 * CIN + c] = 1.0

    in_maps = []
    for k in range(NCORES):
        sl = slice(ISHARD * k, ISHARD * (k + 1))
        pg = pairwise_g[:, sl]                      # (B, 16, N, S, S, G)
        pgr = pg.reshape(B, UPB, 2, N, S, S, GDIM)  # [b,iu,blk2,j,s1,s2,g]
        pgr = pgr.transpose(0, 1, 2, 4, 6, 5, 3)    # [b,iu,blk2,s1,g,s2,j]
        pgr = pgr.reshape(B, UPB, 112, KEYW)

        blob = np.zeros((128, totw), NPBF16)
        blob[0:KROWS, w2c:w2c + 128] = w2
        for b in range(B):
            idx, kw = idxs[b], widths[b]
            nk = len(idx)
            blob[:, yoff[b]:yoff[b] + nk] = ytabf[b][:, idx]
            for iu in range(UPB):
                u = b * UPB + iu
                c0 = ucol[u]
                blob[0:112, c0:c0 + nk] = pgr[b, iu][:, idx]
                blob[112:KROWS, c0:c0 + nk] = lkd[b][:, idx]
                if nk < kw:
                    blob[112:KROWS, c0 + nk:c0 + kw] = -80.0
        in_maps.append({"blob16": blob})
    return in_maps


def _host_finish(s_list, coset_functions, mask, w_lin):
    """Decode per-core (128, 64) den|num columns into the full result."""
    y = np.asarray(coset_functions, dtype=np.float32)
    maskf = np.asarray(mask).astype(np.float32)
    out = np.empty((B, N, S, COUT), np.float32)
    for k in range(NCORES):
        s = np.asarray(s_list[k], np.float32)
        den = s[:, :NUNIT].reshape(2, S, CIN, B, UPB)  # [blk2, s1, c, b, iu]
        num = s[:, NUNIT:].reshape(2, S, CIN, B, UPB)
        den = den.transpose(3, 4, 0, 1, 2).reshape(B, ISHARD, S, CIN)
        num = num.transpose(3, 4, 0, 1, 2).reshape(B, ISHARD, S, CIN)
        sl = slice(ISHARD * k, ISHARD * (k + 1))
        y_q = y[:, sl]
        m_q = maskf[:, sl]
        res = (y_q + num / den) * m_q[..., None]
        out[:, sl] = res @ w_lin.T
    return out


def kernel(pairwise_g, coset_functions, mask, w_y, b_y, w_g, b_g, w_lin):
    pairwise_g = np.asarray(pairwise_g, dtype=np.float32)
    coset_functions = np.asarray(coset_functions, dtype=np.float32)
    mask = np.asarray(mask)
    w_y = np.asarray(w_y, dtype=np.float32)
    w_g = np.asarray(w_g, dtype=np.float32)
    w_lin = np.asarray(w_lin, dtype=np.float32)

    in_maps = _host_prep(pairwise_g, coset_functions, mask, w_y, w_g)
    nc = _get_program()
    res = run_bass_kernel_spmd(nc, in_maps, core_ids=list(range(NCORES)))
    s_list = [r["out_s"] for r in res.results]
    return _host_finish(s_list, coset_functions, mask, w_lin)
